# revision 13
# baseline (speedup 1.0000x reference)
"""MoE layer (B=4,S=2048,D=1024,H=4096,E=8,K=2) on 8 trn2 NeuronCores.

Sharding strategy (hardcoded): expert-parallel with a fixed per-expert
capacity of 2048 tokens (= the perfectly load-balanced share: 8192 tokens
x top-2 / 8 experts). Host computes the gate (logits -> top-2 -> softmax
weights) and dispatches: core e receives up to 2048 tokens routed to
expert e (gathered + transposed + padded to the static capacity), plus
expert e's FFN weights in bf16. Each core runs the expert FFN
(x @ W1 -> gelu -> @ W2, fp32 PSUM accumulation) and scales rows by the
combine weight on-device. Tokens beyond an expert's capacity (the
capacity-factor-1.0 overflow, ~2% of pairs) take the host overflow path
(exact fp32 FFN). Host scatter-adds the weighted per-expert outputs back
into the full [B,S,D] output (the "all-to-all combine"), adding the b2
contribution exactly once per (token, expert) pair.
"""

import sys

for _p in ("/opt/trn_rl_repo", "/root/.axon_site"):
    if _p not in sys.path:
        sys.path.insert(0, _p)

import numpy as np
import ml_dtypes

import concourse.bacc as bacc
import concourse.mybir as mybir
import concourse.tile as tile
from concourse.bass_utils import run_bass_kernel_spmd

BF16 = mybir.dt.bfloat16
F32 = mybir.dt.float32

N_CORES = 8
D = 1024
H = 4096
E = 8

_CACHE: dict = {}
LAST_RESULTS = None  # BassKernelResults of the most recent run (for test.py)
TRACE = False  # test.py can flip this to get an NTFF profile

try:
    from scipy.special import erf as _erf
except ImportError:
    import math

    def _erf(a):
        return np.vectorize(math.erf, otypes=[np.float32])(a)


def _blocks(capT):
    """Split capT tokens into moving-dim blocks: full 512s + one 128-multiple tail."""
    out = []
    t0 = 0
    while t0 < capT:
        tn = min(512, capT - t0)
        out.append((t0, tn))
        t0 += tn
    return out


def _build(capT, with_b1):
    nc = bacc.Bacc("TRN2", target_bir_lowering=False, debug=False,
                   num_devices=N_CORES)

    xT_d = nc.dram_tensor("xT", [8, 128, capT], BF16, kind="ExternalInput")
    w1_d = nc.dram_tensor("w1", [8, 128, H], BF16, kind="ExternalInput")
    w2_d = nc.dram_tensor("w2", [32, 128, D], BF16, kind="ExternalInput")
    wv_d = nc.dram_tensor("wv", [128, capT // 128], F32, kind="ExternalInput")
    if with_b1:
        b1_d = nc.dram_tensor("b1t", [128, 32], F32, kind="ExternalInput")
    # y ships back as bf16: halves the store traffic and the final store's
    # tail latency; the host combine accumulates in fp32 anyway.
    y_d = nc.dram_tensor("y", [capT, D], BF16, kind="ExternalOutput")

    blocks = _blocks(capT)

    with tile.TileContext(nc) as tc:
        with (
            tc.tile_pool(name="weights", bufs=1) as wpool,
            tc.tile_pool(name="xin", bufs=1) as xpool,
            tc.tile_pool(name="hbuf", bufs=2) as hpool,
            tc.tile_pool(name="yout", bufs=3) as ypool,
            tc.tile_pool(name="small", bufs=1) as spool,
            tc.tile_pool(name="ps1", bufs=3, space="PSUM") as ps1pool,
            tc.tile_pool(name="ps2", bufs=2, space="PSUM") as ps2pool,
        ):
            # One trigger per tensor group: DMA triggers serialize ~600ns
            # each on the issuing engine, so use few big 3D-AP DMAs, all on
            # SyncE (ACT must stay free for latency-critical gelu).
            xT_p = xT_d.rearrange("k p c -> p k c")
            w2_p = w2_d.rearrange("j p c -> p j c")

            # token block 0 first so PE can start ASAP, then weights.
            # The first real matmul only needs the k=0 slices of x block 0
            # and of w1 group 0's first 128 columns, so those ship as tiny
            # dedicated DMAs ahead of everything else (~160KB gate instead
            # of ~1.3MB): first MM issues at ~1.5us instead of ~7us.
            # x block 0 streams on the sync HWDGE queue while w1 group 0
            # streams in parallel on the scalar HWDGE queue (separate HW
            # rings), so the two gating transfers don't serialize.
            xsb = {}
            t0, tn = blocks[0]
            xsb[0] = xpool.tile([128, 8, tn], BF16, tag="xT", name="xT0")
            nc.sync.dma_start(xsb[0][:, 0, :], xT_p[:, 0, t0:t0 + tn])

            w1_p = w1_d.rearrange("k p c -> p k c")
            w1g = []
            for g in range(8):
                t = wpool.tile([128, 8, 512], BF16, tag=f"w1g{g}", name=f"w1g{g}")
                w1g.append(t)
            nc.scalar.dma_start(w1g[0][:, 0, :128], w1_p[:, 0, :128])
            nc.sync.dma_start(xsb[0][:, 1:4, :], xT_p[:, 1:4, t0:t0 + tn])
            nc.scalar.dma_start(w1g[0][:, 1:, :128], w1_p[:, 1:, :128])
            nc.sync.dma_start(xsb[0][:, 4:, :], xT_p[:, 4:, t0:t0 + tn])
            nc.scalar.dma_start(w1g[0][:, :, 128:], w1_p[:, :, 128:512])
            if with_b1:
                b1_sb = spool.tile([128, 32], F32)
                nc.scalar.dma_start(b1_sb[:], b1_d[:])
            # remaining W1 column groups of 512 H each (layer-1 consumes in
            # order)
            for g in range(1, 8):
                nc.sync.dma_start(w1g[g][:], w1_p[:, :, g * 512:(g + 1) * 512])
            # W2 in 4 H-groups of 8 k-tiles each
            w2g = []
            for g in range(4):
                t = wpool.tile([128, 8, 1024], BF16, tag=f"w2g{g}", name=f"w2g{g}")
                nc.sync.dma_start(t[:], w2_p[:, g * 8:(g + 1) * 8, :])
                w2g.append(t)

            wv_sb = spool.tile([128, capT // 128], F32)
            nc.sync.dma_start(wv_sb[:], wv_d[:])

            # HAM warm-up: the PE clock sits at 1.2GHz until it has been
            # busy for one ~3.4us activity window. Spend that window on
            # dummy matmuls while the gating DMAs are still in flight, so
            # the real stream starts at 2.4GHz. 34 x N=128 cold MMs ~= 3.6us,
            # landing just before the first x/w1 tiles arrive (~10us).
            warm_src = spool.tile([128, 128], BF16, name="warm_src")
            nc.gpsimd.memset(warm_src[:], 0.0)
            warm_ps = ps1pool.tile([128, 2, 512], F32, tag="ps1",
                                   name="warm_ps", bufs=None)
            for wi in range(34):
                nc.tensor.matmul(
                    warm_ps[:64, 0, :128], warm_src[:, :64], warm_src[:],
                    start=True, stop=True, skip_group_check=True)

            for blk, (t0, tn) in enumerate(blocks):
                if blk not in xsb:
                    xsb[blk] = xpool.tile([128, 8, tn], BF16, tag="xT", name=f"xT{blk}")
                    nc.sync.dma_start(xsb[blk][:], xT_p[:, :, t0:t0 + tn])
                xt = xsb[blk]

                # ---- layer 1: hT[m*128:(m+1)*128, :tn] for 32 H-tiles ----
                hT = hpool.tile([128, 32, 512], BF16, tag="hT", name=f"hT{blk}")
                for mg in range(16):
                    ps1 = ps1pool.tile([128, 2, 512], F32, tag="ps1", name=f"ps1_{blk}_{mg}")
                    for mj in range(2):
                        m = mg * 2 + mj
                        lg, lo = m // 4, m % 4
                        for k in range(8):
                            nc.tensor.matmul(
                                ps1[:, mj, :tn],
                                w1g[lg][:, k, lo * 128:(lo + 1) * 128],
                                xt[:, k, :tn],
                                start=(k == 0), stop=(k == 7),
                            )
                    if with_b1:
                        for mj in range(2):
                            m = mg * 2 + mj
                            nc.scalar.activation(
                                hT[:, m, :tn], ps1[:, mj, :tn],
                                mybir.ActivationFunctionType.Gelu,
                                bias=b1_sb[:, m:m + 1],
                            )
                    else:
                        nc.scalar.activation(
                            hT[:, mg * 2:mg * 2 + 2, :tn], ps1[:, :, :tn],
                            mybir.ActivationFunctionType.Gelu,
                        )

                # ---- layer 2: y[t0+tm*128 ..., :] = hT.T @ W2, scaled ----
                for tm in range(tn // 128):
                    col = t0 // 128 + tm
                    rows = slice(t0 + tm * 128, t0 + (tm + 1) * 128)
                    for dn in range(2):
                        ps2 = ps2pool.tile([128, 512], F32, tag="ps2",
                                           name=f"ps2_{blk}_{tm}_{dn}")
                        for h in range(32):
                            nc.tensor.matmul(
                                ps2[:, :],
                                hT[:, h, tm * 128:(tm + 1) * 128],
                                w2g[h // 8][:, h % 8, dn * 512:(dn + 1) * 512],
                                start=(h == 0), stop=(h == 31),
                            )
                        yt = ypool.tile([128, 512], BF16, tag="yt",
                                        name=f"yt_{blk}_{tm}_{dn}")
                        nc.vector.tensor_scalar_mul(
                            yt[:], ps2[:], wv_sb[:, col:col + 1])
                        if blk == len(blocks) - 1 and dn == 1:
                            nc.scalar.dma_start(
                                y_d[rows, 512:1024], yt[:])
                        else:
                            nc.sync.dma_start(
                                y_d[rows, dn * 512:(dn + 1) * 512], yt[:])

    nc.compile()
    return nc


def _route(x_flat, Wg, bg):
    """Host gate: returns per-expert (token_idx, combine_weight)."""
    logits = x_flat @ Wg.astype(np.float32) + bg.astype(np.float32)
    T = logits.shape[0]
    ar = np.arange(T)
    top1 = np.argmax(logits, axis=1)
    l2 = logits.copy()
    l2[ar, top1] = -np.inf
    top2 = np.argmax(l2, axis=1)
    v1 = logits[ar, top1]
    v2 = logits[ar, top2]
    # softmax over the two selected logits (v1 >= v2)
    e2 = np.exp(v2 - v1)
    s = 1.0 + e2
    wt1 = (1.0 / s).astype(np.float32)
    wt2 = (e2 / s).astype(np.float32)
    idx, wgt = [], []
    for e in range(E):
        m1 = top1 == e
        m2 = top2 == e
        ii = np.concatenate([ar[m1], ar[m2]])
        ww = np.concatenate([wt1[m1], wt2[m2]])
        order = np.argsort(ii, kind="stable")
        idx.append(ii[order])
        wgt.append(ww[order])
    return idx, wgt


def kernel(x, Wg, bg, W1, b1, W2, b2, _trace=None):
    global LAST_RESULTS
    x = np.asarray(x, dtype=np.float32)
    Wg = np.asarray(Wg, dtype=np.float32)
    bg = np.asarray(bg, dtype=np.float32)
    W1 = np.asarray(W1, dtype=np.float32)
    b1 = np.asarray(b1, dtype=np.float32)
    W2 = np.asarray(W2, dtype=np.float32)
    b2 = np.asarray(b2, dtype=np.float32)

    B, S, _D = x.shape
    T = B * S
    x_flat = np.ascontiguousarray(x.reshape(T, _D))

    idx, wgt = _route(x_flat, Wg, bg)
    counts = [len(i) for i in idx]
    # Fixed capacity = the load-balanced share (capacity factor 1.0).
    # Every core computes capT tokens regardless, so padding all cores to
    # the max expert count just burns PE time on zeros; overflow beyond
    # capT takes the exact host path instead.
    capT = min(2048, max(512, -(-max(counts) // 128) * 128))
    dev_counts = [min(c, capT) for c in counts]

    with_b1 = bool(np.any(b1))
    key = (capT, with_b1)
    if key not in _CACHE:
        _CACHE[key] = _build(capT, with_b1)
    nc = _CACHE[key]

    bf = ml_dtypes.bfloat16
    in_maps = []
    for e in range(E):
        cnt = dev_counts[e]
        xT = np.zeros((D, capT), dtype=bf)
        if cnt:
            xT[:, :cnt] = x_flat[idx[e][:cnt]].T
        wv = np.zeros((capT // 128, 128), dtype=np.float32)
        if cnt:
            wv.reshape(-1)[:cnt] = wgt[e][:cnt]
        m = {
            "xT": np.ascontiguousarray(xT.reshape(8, 128, capT)),
            "w1": np.ascontiguousarray(W1[e].astype(bf).reshape(8, 128, H)),
            "w2": np.ascontiguousarray(W2[e].astype(bf).reshape(32, 128, D)),
            "wv": np.ascontiguousarray(wv.T),
        }
        if with_b1:
            m["b1t"] = np.ascontiguousarray(b1[e].reshape(32, 128).T)
        in_maps.append(m)

    do_trace = TRACE if _trace is None else _trace
    res = run_bass_kernel_spmd(nc, in_maps, list(range(N_CORES)),
                               trace=do_trace)
    LAST_RESULTS = res

    out = np.zeros((T, D), dtype=np.float32)
    for e in range(E):
        cnt = dev_counts[e]
        if not cnt:
            continue
        ye = res.results[e]["y"][:cnt].astype(np.float32)
        if np.any(b2[e]):
            ye = ye + np.outer(wgt[e][:cnt], b2[e])
        out[idx[e][:cnt]] += ye

    # Host overflow path: exact fp32 FFN for tokens beyond expert capacity.
    for e in range(E):
        if counts[e] <= capT:
            continue
        oi = idx[e][capT:]
        ow = wgt[e][capT:]
        h = x_flat[oi] @ W1[e] + b1[e]
        h = 0.5 * h * (1.0 + _erf(h * np.float32(0.7071067811865476)))
        ye = h @ W2[e] + b2[e]
        out[oi] += ow[:, None] * ye

    return out.reshape(B, S, D)



# revision 14
# speedup vs baseline: 1.0044x; 1.0044x over previous
"""MoE layer (B=4,S=2048,D=1024,H=4096,E=8,K=2) on 8 trn2 NeuronCores.

Sharding strategy (hardcoded): expert-parallel with a fixed per-expert
capacity of 2048 tokens (= the perfectly load-balanced share: 8192 tokens
x top-2 / 8 experts). Host computes the gate (logits -> top-2 -> softmax
weights) and dispatches: core e receives up to 2048 tokens routed to
expert e (gathered + transposed + padded to the static capacity), plus
expert e's FFN weights in bf16. Each core runs the expert FFN
(x @ W1 -> gelu -> @ W2, fp32 PSUM accumulation) and scales rows by the
combine weight on-device. Tokens beyond an expert's capacity (the
capacity-factor-1.0 overflow, ~2% of pairs) take the host overflow path
(exact fp32 FFN). Host scatter-adds the weighted per-expert outputs back
into the full [B,S,D] output (the "all-to-all combine"), adding the b2
contribution exactly once per (token, expert) pair.

All device inputs are host-packed into SBUF layout ([128 partitions,
contiguous free dim]) so every DMA is 128 descriptors of >=1KB contiguous
runs: descriptor-generation (DIRECT2D) drops from ~5.5us to ~0.7us per
transfer and the transfers run at line rate.
"""

import sys

for _p in ("/opt/trn_rl_repo", "/root/.axon_site"):
    if _p not in sys.path:
        sys.path.insert(0, _p)

import numpy as np
import ml_dtypes

import concourse.bacc as bacc
import concourse.mybir as mybir
import concourse.tile as tile
from concourse.bass_utils import run_bass_kernel_spmd

BF16 = mybir.dt.bfloat16
F32 = mybir.dt.float32

N_CORES = 8
D = 1024
H = 4096
E = 8

_CACHE: dict = {}
LAST_RESULTS = None  # BassKernelResults of the most recent run (for test.py)
TRACE = False  # test.py can flip this to get an NTFF profile

try:
    from scipy.special import erf as _erf
except ImportError:
    import math

    def _erf(a):
        return np.vectorize(math.erf, otypes=[np.float32])(a)


def _build(capT, with_b1):
    nc = bacc.Bacc("TRN2", target_bir_lowering=False, debug=False,
                   num_devices=N_CORES)

    nblk = capT // 512
    # Packed layouts (built host-side):
    #   xp[p, blk, k, c]  = x_flat[token blk*512+c, d=k*128+p]
    #   w1p[p, g, k, c]   = W1[d=k*128+p, h=g*512+c]
    #   w2p[p, g, j, c]   = W2[h=(g*8+j)*128+p, d=c]
    xp_d = nc.dram_tensor("xp", [128, nblk, 8, 512], BF16, kind="ExternalInput")
    w1p_d = nc.dram_tensor("w1p", [128, 8, 8, 512], BF16, kind="ExternalInput")
    w2p_d = nc.dram_tensor("w2p", [128, 4, 8, 1024], BF16, kind="ExternalInput")
    wv_d = nc.dram_tensor("wv", [128, capT // 128], F32, kind="ExternalInput")
    if with_b1:
        b1_d = nc.dram_tensor("b1t", [128, 32], F32, kind="ExternalInput")
    # y ships back as bf16: halves the store traffic and the final store's
    # tail latency; the host combine accumulates in fp32 anyway.
    y_d = nc.dram_tensor("y", [capT, D], BF16, kind="ExternalOutput")

    with tile.TileContext(nc) as tc:
        with (
            tc.tile_pool(name="weights", bufs=1) as wpool,
            tc.tile_pool(name="xin", bufs=1) as xpool,
            tc.tile_pool(name="hbuf", bufs=2) as hpool,
            tc.tile_pool(name="yout", bufs=3) as ypool,
            tc.tile_pool(name="small", bufs=1) as spool,
            tc.tile_pool(name="ps1", bufs=3, space="PSUM") as ps1pool,
            tc.tile_pool(name="ps2", bufs=2, space="PSUM") as ps2pool,
        ):
            # Gating transfers for the very first matmul (x block0 k=0 and
            # w1 group0 k=0) ship first, split across the sync and scalar
            # HWDGE queues so they don't serialize behind each other.
            xsb = {}
            xsb[0] = xpool.tile([128, 8, 512], BF16, tag="xT", name="xT0")
            nc.sync.dma_start(xsb[0][:, 0, :], xp_d[:, 0, 0])

            w1g = []
            for g in range(8):
                t = wpool.tile([128, 8, 512], BF16, tag=f"w1g{g}", name=f"w1g{g}")
                w1g.append(t)
            nc.scalar.dma_start(w1g[0][:, 0, :], w1p_d[:, 0, 0])
            nc.sync.dma_start(xsb[0][:, 1:4, :], xp_d[:, 0, 1:4])
            nc.scalar.dma_start(w1g[0][:, 1:, :], w1p_d[:, 0, 1:])
            nc.sync.dma_start(xsb[0][:, 4:, :], xp_d[:, 0, 4:])
            if with_b1:
                b1_sb = spool.tile([128, 32], F32)
                nc.scalar.dma_start(b1_sb[:], b1_d[:])
            for g in range(1, 8):
                nc.sync.dma_start(w1g[g][:], w1p_d[:, g])
            w2g = []
            for g in range(4):
                t = wpool.tile([128, 8, 1024], BF16, tag=f"w2g{g}", name=f"w2g{g}")
                nc.sync.dma_start(t[:], w2p_d[:, g])
                w2g.append(t)

            wv_sb = spool.tile([128, capT // 128], F32)
            nc.sync.dma_start(wv_sb[:], wv_d[:])

            # HAM warm-up: the PE clock sits at 1.2GHz until it has been
            # busy ~3.4us. Spend that window on dummy matmuls while the
            # gating DMAs are in flight; sized so the burst ends right as
            # the first real tiles land (~9.5us), handing off without an
            # idle gap (an idle gap >3.4us would re-throttle the clock).
            warm_src = spool.tile([128, 128], BF16, name="warm_src")
            nc.gpsimd.memset(warm_src[:], 0.0)
            warm_ps = ps1pool.tile([128, 2, 512], F32, tag="ps1",
                                   name="warm_ps", bufs=None)
            for wi in range(22):
                nc.tensor.matmul(
                    warm_ps[:64, 0, :128], warm_src[:, :64], warm_src[:],
                    start=True, stop=True, skip_group_check=True)

            for blk in range(nblk):
                t0, tn = blk * 512, 512
                if blk not in xsb:
                    xsb[blk] = xpool.tile([128, 8, 512], BF16, tag="xT",
                                          name=f"xT{blk}")
                    nc.sync.dma_start(xsb[blk][:], xp_d[:, blk])
                xt = xsb[blk]

                # ---- layer 1: hT[m*128:(m+1)*128, :tn] for 32 H-tiles ----
                hT = hpool.tile([128, 32, 512], BF16, tag="hT", name=f"hT{blk}")
                for mg in range(16):
                    ps1 = ps1pool.tile([128, 2, 512], F32, tag="ps1",
                                       name=f"ps1_{blk}_{mg}")
                    for mj in range(2):
                        m = mg * 2 + mj
                        lg, lo = m // 4, m % 4
                        for k in range(8):
                            nc.tensor.matmul(
                                ps1[:, mj, :tn],
                                w1g[lg][:, k, lo * 128:(lo + 1) * 128],
                                xt[:, k, :tn],
                                start=(k == 0), stop=(k == 7),
                            )
                    if with_b1:
                        for mj in range(2):
                            m = mg * 2 + mj
                            nc.scalar.activation(
                                hT[:, m, :tn], ps1[:, mj, :tn],
                                mybir.ActivationFunctionType.Gelu,
                                bias=b1_sb[:, m:m + 1],
                            )
                    else:
                        nc.scalar.activation(
                            hT[:, mg * 2:mg * 2 + 2, :tn], ps1[:, :, :tn],
                            mybir.ActivationFunctionType.Gelu,
                        )

                # ---- layer 2: y[t0+tm*128 ..., :] = hT.T @ W2, scaled ----
                for tm in range(tn // 128):
                    col = t0 // 128 + tm
                    rows = slice(t0 + tm * 128, t0 + (tm + 1) * 128)
                    for dn in range(2):
                        ps2 = ps2pool.tile([128, 512], F32, tag="ps2",
                                           name=f"ps2_{blk}_{tm}_{dn}")
                        for h in range(32):
                            nc.tensor.matmul(
                                ps2[:, :],
                                hT[:, h, tm * 128:(tm + 1) * 128],
                                w2g[h // 8][:, h % 8, dn * 512:(dn + 1) * 512],
                                start=(h == 0), stop=(h == 31),
                            )
                        yt = ypool.tile([128, 512], BF16, tag="yt",
                                        name=f"yt_{blk}_{tm}_{dn}")
                        nc.vector.tensor_scalar_mul(
                            yt[:], ps2[:], wv_sb[:, col:col + 1])
                        if blk == nblk - 1 and dn == 1:
                            nc.scalar.dma_start(
                                y_d[rows, 512:1024], yt[:])
                        else:
                            nc.sync.dma_start(
                                y_d[rows, dn * 512:(dn + 1) * 512], yt[:])

    nc.compile()
    return nc


def _route(x_flat, Wg, bg):
    """Host gate: returns per-expert (token_idx, combine_weight)."""
    logits = x_flat @ Wg.astype(np.float32) + bg.astype(np.float32)
    T = logits.shape[0]
    ar = np.arange(T)
    top1 = np.argmax(logits, axis=1)
    l2 = logits.copy()
    l2[ar, top1] = -np.inf
    top2 = np.argmax(l2, axis=1)
    v1 = logits[ar, top1]
    v2 = logits[ar, top2]
    # softmax over the two selected logits (v1 >= v2)
    e2 = np.exp(v2 - v1)
    s = 1.0 + e2
    wt1 = (1.0 / s).astype(np.float32)
    wt2 = (e2 / s).astype(np.float32)
    idx, wgt = [], []
    for e in range(E):
        m1 = top1 == e
        m2 = top2 == e
        ii = np.concatenate([ar[m1], ar[m2]])
        ww = np.concatenate([wt1[m1], wt2[m2]])
        order = np.argsort(ii, kind="stable")
        idx.append(ii[order])
        wgt.append(ww[order])
    return idx, wgt


def kernel(x, Wg, bg, W1, b1, W2, b2, _trace=None):
    global LAST_RESULTS
    x = np.asarray(x, dtype=np.float32)
    Wg = np.asarray(Wg, dtype=np.float32)
    bg = np.asarray(bg, dtype=np.float32)
    W1 = np.asarray(W1, dtype=np.float32)
    b1 = np.asarray(b1, dtype=np.float32)
    W2 = np.asarray(W2, dtype=np.float32)
    b2 = np.asarray(b2, dtype=np.float32)

    B, S, _D = x.shape
    T = B * S
    x_flat = np.ascontiguousarray(x.reshape(T, _D))

    idx, wgt = _route(x_flat, Wg, bg)
    counts = [len(i) for i in idx]
    # Fixed capacity = the load-balanced share (capacity factor 1.0).
    # Every core computes capT tokens regardless, so padding all cores to
    # the max expert count just burns PE time on zeros; overflow beyond
    # capT takes the exact host path instead.
    capT = min(2048, max(512, -(-max(counts) // 512) * 512))
    dev_counts = [min(c, capT) for c in counts]
    nblk = capT // 512

    with_b1 = bool(np.any(b1))
    key = (capT, with_b1)
    if key not in _CACHE:
        _CACHE[key] = _build(capT, with_b1)
    nc = _CACHE[key]

    bf = ml_dtypes.bfloat16
    in_maps = []
    for e in range(E):
        cnt = dev_counts[e]
        xT = np.zeros((D, capT), dtype=bf)
        if cnt:
            xT[:, :cnt] = x_flat[idx[e][:cnt]].T
        xp = xT.reshape(8, 128, nblk, 512).transpose(1, 2, 0, 3)
        w1p = W1[e].astype(bf).reshape(8, 128, 8, 512).transpose(1, 2, 0, 3)
        w2p = W2[e].astype(bf).reshape(4, 8, 128, 1024).transpose(2, 0, 1, 3)
        wv = np.zeros((capT // 128, 128), dtype=np.float32)
        if cnt:
            wv.reshape(-1)[:cnt] = wgt[e][:cnt]
        m = {
            "xp": np.ascontiguousarray(xp),
            "w1p": np.ascontiguousarray(w1p),
            "w2p": np.ascontiguousarray(w2p),
            "wv": np.ascontiguousarray(wv.T),
        }
        if with_b1:
            m["b1t"] = np.ascontiguousarray(b1[e].reshape(32, 128).T)
        in_maps.append(m)

    do_trace = TRACE if _trace is None else _trace
    res = run_bass_kernel_spmd(nc, in_maps, list(range(N_CORES)),
                               trace=do_trace)
    LAST_RESULTS = res

    out = np.zeros((T, D), dtype=np.float32)
    for e in range(E):
        cnt = dev_counts[e]
        if not cnt:
            continue
        ye = res.results[e]["y"][:cnt].astype(np.float32)
        if np.any(b2[e]):
            ye = ye + np.outer(wgt[e][:cnt], b2[e])
        out[idx[e][:cnt]] += ye

    # Host overflow path: exact fp32 FFN for tokens beyond expert capacity.
    for e in range(E):
        if counts[e] <= capT:
            continue
        oi = idx[e][capT:]
        ow = wgt[e][capT:]
        h = x_flat[oi] @ W1[e] + b1[e]
        h = 0.5 * h * (1.0 + _erf(h * np.float32(0.7071067811865476)))
        ye = h @ W2[e] + b2[e]
        out[oi] += ow[:, None] * ye

    return out.reshape(B, S, D)


# revision 16
# speedup vs baseline: 1.0047x; 1.0002x over previous
"""MoE layer (B=4,S=2048,D=1024,H=4096,E=8,K=2) on 8 trn2 NeuronCores.

Sharding strategy (hardcoded): expert-parallel with a fixed per-expert
capacity of 2048 tokens (= the perfectly load-balanced share: 8192 tokens
x top-2 / 8 experts). Host computes the gate (logits -> top-2 -> softmax
weights) and dispatches: core e receives up to 2048 tokens routed to
expert e (gathered + transposed + padded to the static capacity), plus
expert e's FFN weights in bf16. Each core runs the expert FFN
(x @ W1 -> gelu -> @ W2, fp32 PSUM accumulation) and scales rows by the
combine weight on-device. Tokens beyond an expert's capacity (the
capacity-factor-1.0 overflow, ~2% of pairs) take the host overflow path
(exact fp32 FFN). Host scatter-adds the weighted per-expert outputs back
into the full [B,S,D] output (the "all-to-all combine"), adding the b2
contribution exactly once per (token, expert) pair.

All device inputs are host-packed into SBUF layout ([128 partitions,
contiguous free dim]) so every DMA is 128 descriptors of >=1KB contiguous
runs: descriptor-generation (DIRECT2D) drops from ~5.5us to ~0.7us per
transfer and the transfers run at line rate.
"""

import sys

for _p in ("/opt/trn_rl_repo", "/root/.axon_site"):
    if _p not in sys.path:
        sys.path.insert(0, _p)

import numpy as np
import ml_dtypes

import concourse.bacc as bacc
import concourse.mybir as mybir
import concourse.tile as tile
from concourse.bass_utils import run_bass_kernel_spmd

BF16 = mybir.dt.bfloat16
F32 = mybir.dt.float32

N_CORES = 8
D = 1024
H = 4096
E = 8

_CACHE: dict = {}
LAST_RESULTS = None  # BassKernelResults of the most recent run (for test.py)
TRACE = False  # test.py can flip this to get an NTFF profile

try:
    from scipy.special import erf as _erf
except ImportError:
    import math

    def _erf(a):
        return np.vectorize(math.erf, otypes=[np.float32])(a)


def _build(capT, with_b1):
    nc = bacc.Bacc("TRN2", target_bir_lowering=False, debug=False,
                   num_devices=N_CORES)

    nblk = capT // 512
    # Packed layouts (built host-side):
    #   xp[p, blk, k, c]  = x_flat[token blk*512+c, d=k*128+p]
    #   w1p[p, g, k, c]   = W1[d=k*128+p, h=g*512+c]
    #   w2p[p, g, j, c]   = W2[h=(g*8+j)*128+p, d=c]
    xp_d = nc.dram_tensor("xp", [128, nblk, 8, 512], BF16, kind="ExternalInput")
    w1p_d = nc.dram_tensor("w1p", [128, 8, 8, 512], BF16, kind="ExternalInput")
    w2p_d = nc.dram_tensor("w2p", [128, 4, 8, 1024], BF16, kind="ExternalInput")
    wv_d = nc.dram_tensor("wv", [128, capT // 128], F32, kind="ExternalInput")
    if with_b1:
        b1_d = nc.dram_tensor("b1t", [128, 32], F32, kind="ExternalInput")
    # y ships back as bf16: halves the store traffic and the final store's
    # tail latency; the host combine accumulates in fp32 anyway.
    y_d = nc.dram_tensor("y", [capT, D], BF16, kind="ExternalOutput")

    with tile.TileContext(nc) as tc:
        with (
            tc.tile_pool(name="weights", bufs=1) as wpool,
            tc.tile_pool(name="xin", bufs=1) as xpool,
            tc.tile_pool(name="hbuf", bufs=2) as hpool,
            tc.tile_pool(name="yout", bufs=3) as ypool,
            tc.tile_pool(name="small", bufs=1) as spool,
            tc.tile_pool(name="ps1", bufs=3, space="PSUM") as ps1pool,
            tc.tile_pool(name="ps2", bufs=2, space="PSUM") as ps2pool,
        ):
            # Gating transfers for the very first matmul (x block0 k=0 and
            # w1 group0 k=0) ship first, split across the sync and scalar
            # HWDGE queues so they don't serialize behind each other.
            xsb = {}
            xsb[0] = xpool.tile([128, 8, 512], BF16, tag="xT", name="xT0")
            nc.sync.dma_start(xsb[0][:], xp_d[:, 0])

            w1g = []
            for g in range(8):
                t = wpool.tile([128, 8, 512], BF16, tag=f"w1g{g}", name=f"w1g{g}")
                w1g.append(t)
            nc.scalar.dma_start(w1g[0][:], w1p_d[:, 0])
            if with_b1:
                b1_sb = spool.tile([128, 32], F32)
                nc.scalar.dma_start(b1_sb[:], b1_d[:])
            for g in range(1, 8):
                nc.sync.dma_start(w1g[g][:], w1p_d[:, g])
            w2g = []
            for g in range(4):
                t = wpool.tile([128, 8, 1024], BF16, tag=f"w2g{g}", name=f"w2g{g}")
                nc.sync.dma_start(t[:], w2p_d[:, g])
                w2g.append(t)

            wv_sb = spool.tile([128, capT // 128], F32)
            nc.sync.dma_start(wv_sb[:], wv_d[:])

            # HAM warm-up: the PE clock sits at 1.2GHz until it has been
            # busy ~3.4us. Spend that window on dummy matmuls while the
            # gating DMAs are in flight; sized so the burst ends right as
            # the first real tiles land (~9.5us), handing off without an
            # idle gap (an idle gap >3.4us would re-throttle the clock).
            warm_src = spool.tile([128, 128], BF16, name="warm_src")
            nc.gpsimd.memset(warm_src[:], 0.0)
            warm_ps = ps1pool.tile([128, 2, 512], F32, tag="ps1",
                                   name="warm_ps", bufs=None)
            for wi in range(56):
                nc.tensor.matmul(
                    warm_ps[:64, 0, :128], warm_src[:, :64], warm_src[:],
                    start=True, stop=True, skip_group_check=True)

            for blk in range(nblk):
                t0, tn = blk * 512, 512
                if blk not in xsb:
                    xsb[blk] = xpool.tile([128, 8, 512], BF16, tag="xT",
                                          name=f"xT{blk}")
                    nc.sync.dma_start(xsb[blk][:], xp_d[:, blk])
                xt = xsb[blk]

                # ---- layer 1: hT[m*128:(m+1)*128, :tn] for 32 H-tiles ----
                hT = hpool.tile([128, 32, 512], BF16, tag="hT", name=f"hT{blk}")
                for mg in range(16):
                    ps1 = ps1pool.tile([128, 2, 512], F32, tag="ps1",
                                       name=f"ps1_{blk}_{mg}")
                    for mj in range(2):
                        m = mg * 2 + mj
                        lg, lo = m // 4, m % 4
                        for k in range(8):
                            nc.tensor.matmul(
                                ps1[:, mj, :tn],
                                w1g[lg][:, k, lo * 128:(lo + 1) * 128],
                                xt[:, k, :tn],
                                start=(k == 0), stop=(k == 7),
                            )
                    if with_b1:
                        for mj in range(2):
                            m = mg * 2 + mj
                            nc.scalar.activation(
                                hT[:, m, :tn], ps1[:, mj, :tn],
                                mybir.ActivationFunctionType.Gelu,
                                bias=b1_sb[:, m:m + 1],
                            )
                    else:
                        nc.scalar.activation(
                            hT[:, mg * 2:mg * 2 + 2, :tn], ps1[:, :, :tn],
                            mybir.ActivationFunctionType.Gelu,
                        )

                # ---- layer 2: y[t0+tm*128 ..., :] = hT.T @ W2, scaled ----
                for tm in range(tn // 128):
                    col = t0 // 128 + tm
                    rows = slice(t0 + tm * 128, t0 + (tm + 1) * 128)
                    for dn in range(2):
                        ps2 = ps2pool.tile([128, 512], F32, tag="ps2",
                                           name=f"ps2_{blk}_{tm}_{dn}")
                        for h in range(32):
                            nc.tensor.matmul(
                                ps2[:, :],
                                hT[:, h, tm * 128:(tm + 1) * 128],
                                w2g[h // 8][:, h % 8, dn * 512:(dn + 1) * 512],
                                start=(h == 0), stop=(h == 31),
                            )
                        yt = ypool.tile([128, 512], BF16, tag="yt",
                                        name=f"yt_{blk}_{tm}_{dn}")
                        nc.vector.tensor_scalar_mul(
                            yt[:], ps2[:], wv_sb[:, col:col + 1])
                        if blk == nblk - 1 and dn == 1:
                            nc.scalar.dma_start(
                                y_d[rows, 512:1024], yt[:])
                        else:
                            nc.sync.dma_start(
                                y_d[rows, dn * 512:(dn + 1) * 512], yt[:])

    nc.compile()
    return nc


def _route(x_flat, Wg, bg):
    """Host gate: returns per-expert (token_idx, combine_weight)."""
    logits = x_flat @ Wg.astype(np.float32) + bg.astype(np.float32)
    T = logits.shape[0]
    ar = np.arange(T)
    top1 = np.argmax(logits, axis=1)
    l2 = logits.copy()
    l2[ar, top1] = -np.inf
    top2 = np.argmax(l2, axis=1)
    v1 = logits[ar, top1]
    v2 = logits[ar, top2]
    # softmax over the two selected logits (v1 >= v2)
    e2 = np.exp(v2 - v1)
    s = 1.0 + e2
    wt1 = (1.0 / s).astype(np.float32)
    wt2 = (e2 / s).astype(np.float32)
    idx, wgt = [], []
    for e in range(E):
        m1 = top1 == e
        m2 = top2 == e
        ii = np.concatenate([ar[m1], ar[m2]])
        ww = np.concatenate([wt1[m1], wt2[m2]])
        order = np.argsort(ii, kind="stable")
        idx.append(ii[order])
        wgt.append(ww[order])
    return idx, wgt


def kernel(x, Wg, bg, W1, b1, W2, b2, _trace=None):
    global LAST_RESULTS
    x = np.asarray(x, dtype=np.float32)
    Wg = np.asarray(Wg, dtype=np.float32)
    bg = np.asarray(bg, dtype=np.float32)
    W1 = np.asarray(W1, dtype=np.float32)
    b1 = np.asarray(b1, dtype=np.float32)
    W2 = np.asarray(W2, dtype=np.float32)
    b2 = np.asarray(b2, dtype=np.float32)

    B, S, _D = x.shape
    T = B * S
    x_flat = np.ascontiguousarray(x.reshape(T, _D))

    idx, wgt = _route(x_flat, Wg, bg)
    counts = [len(i) for i in idx]
    # Fixed capacity = the load-balanced share (capacity factor 1.0).
    # Every core computes capT tokens regardless, so padding all cores to
    # the max expert count just burns PE time on zeros; overflow beyond
    # capT takes the exact host path instead.
    capT = min(2048, max(512, -(-max(counts) // 512) * 512))
    dev_counts = [min(c, capT) for c in counts]
    nblk = capT // 512

    with_b1 = bool(np.any(b1))
    key = (capT, with_b1)
    if key not in _CACHE:
        _CACHE[key] = _build(capT, with_b1)
    nc = _CACHE[key]

    bf = ml_dtypes.bfloat16
    in_maps = []
    for e in range(E):
        cnt = dev_counts[e]
        xT = np.zeros((D, capT), dtype=bf)
        if cnt:
            xT[:, :cnt] = x_flat[idx[e][:cnt]].T
        xp = xT.reshape(8, 128, nblk, 512).transpose(1, 2, 0, 3)
        w1p = W1[e].astype(bf).reshape(8, 128, 8, 512).transpose(1, 2, 0, 3)
        w2p = W2[e].astype(bf).reshape(4, 8, 128, 1024).transpose(2, 0, 1, 3)
        wv = np.zeros((capT // 128, 128), dtype=np.float32)
        if cnt:
            wv.reshape(-1)[:cnt] = wgt[e][:cnt]
        m = {
            "xp": np.ascontiguousarray(xp),
            "w1p": np.ascontiguousarray(w1p),
            "w2p": np.ascontiguousarray(w2p),
            "wv": np.ascontiguousarray(wv.T),
        }
        if with_b1:
            m["b1t"] = np.ascontiguousarray(b1[e].reshape(32, 128).T)
        in_maps.append(m)

    do_trace = TRACE if _trace is None else _trace
    res = run_bass_kernel_spmd(nc, in_maps, list(range(N_CORES)),
                               trace=do_trace)
    LAST_RESULTS = res

    out = np.zeros((T, D), dtype=np.float32)
    for e in range(E):
        cnt = dev_counts[e]
        if not cnt:
            continue
        ye = res.results[e]["y"][:cnt].astype(np.float32)
        if np.any(b2[e]):
            ye = ye + np.outer(wgt[e][:cnt], b2[e])
        out[idx[e][:cnt]] += ye

    # Host overflow path: exact fp32 FFN for tokens beyond expert capacity.
    for e in range(E):
        if counts[e] <= capT:
            continue
        oi = idx[e][capT:]
        ow = wgt[e][capT:]
        h = x_flat[oi] @ W1[e] + b1[e]
        h = 0.5 * h * (1.0 + _erf(h * np.float32(0.7071067811865476)))
        ye = h @ W2[e] + b2[e]
        out[oi] += ow[:, None] * ye

    return out.reshape(B, S, D)


# revision 22
# speedup vs baseline: 1.0142x; 1.0095x over previous
"""MoE layer (B=4,S=2048,D=1024,H=4096,E=8,K=2) on 8 trn2 NeuronCores.

Sharding strategy (hardcoded): expert-parallel with a fixed per-expert
capacity of 2048 tokens (= the perfectly load-balanced share: 8192 tokens
x top-2 / 8 experts). Host computes the gate (logits -> top-2 -> softmax
weights) and dispatches: core e receives up to 2048 tokens routed to
expert e (gathered + transposed + padded to the static capacity), plus
expert e's FFN weights in bf16. Each core runs the expert FFN
(x @ W1 -> gelu -> @ W2, fp32 PSUM accumulation) and scales rows by the
combine weight on-device. Tokens beyond an expert's capacity (the
capacity-factor-1.0 overflow, ~2% of pairs) take the host overflow path
(exact fp32 FFN). Host scatter-adds the weighted per-expert outputs back
into the full [B,S,D] output (the "all-to-all combine"), adding the b2
contribution exactly once per (token, expert) pair.

All device inputs are host-packed into SBUF layout ([128 partitions,
contiguous free dim]) so every DMA is 128 descriptors of >=1KB contiguous
runs: descriptor-generation (DIRECT2D) drops from ~5.5us to ~0.7us per
transfer and the transfers run at line rate.
"""

import sys

for _p in ("/opt/trn_rl_repo", "/root/.axon_site"):
    if _p not in sys.path:
        sys.path.insert(0, _p)

import numpy as np
import ml_dtypes

import concourse.bacc as bacc
import concourse.mybir as mybir
import concourse.tile as tile
from concourse.bass_utils import run_bass_kernel_spmd

BF16 = mybir.dt.bfloat16
F32 = mybir.dt.float32

N_CORES = 8
D = 1024
H = 4096
E = 8

_CACHE: dict = {}
LAST_RESULTS = None  # BassKernelResults of the most recent run (for test.py)
TRACE = False  # test.py can flip this to get an NTFF profile

try:
    from scipy.special import erf as _erf
except ImportError:
    import math

    def _erf(a):
        return np.vectorize(math.erf, otypes=[np.float32])(a)


def _build(capT, with_b1):
    nc = bacc.Bacc("TRN2", target_bir_lowering=False, debug=False,
                   num_devices=N_CORES)

    nblk = capT // 512
    # Packed layouts (built host-side):
    #   xp[p, blk, k, c]     = x_flat[token blk*512+c, d=k*128+p]
    #   w1p[p, g, hf, k, c]  = W1[d=k*128+p, h=g*512+hf*256+c]
    #   w2p[p, g, j, c]      = W2[h=(g*8+j)*128+p, d=c]
    xp_d = nc.dram_tensor("xp", [128, nblk, 8, 512], BF16, kind="ExternalInput")
    w1p_d = nc.dram_tensor("w1p", [128, 8, 2, 8, 256], BF16,
                           kind="ExternalInput")
    w2p_d = nc.dram_tensor("w2p", [128, 4, 8, 1024], BF16, kind="ExternalInput")
    wv_d = nc.dram_tensor("wv", [128, capT // 128], F32, kind="ExternalInput")
    if with_b1:
        b1_d = nc.dram_tensor("b1t", [128, 32], F32, kind="ExternalInput")
    # y ships back as bf16: halves the store traffic and the final store's
    # tail latency; the host combine accumulates in fp32 anyway.
    y_d = nc.dram_tensor("y", [capT, D], BF16, kind="ExternalOutput")

    with tile.TileContext(nc) as tc:
        with (
            tc.tile_pool(name="weights", bufs=1) as wpool,
            tc.tile_pool(name="xin", bufs=1) as xpool,
            tc.tile_pool(name="hbuf", bufs=2) as hpool,
            tc.tile_pool(name="yout", bufs=3) as ypool,
            tc.tile_pool(name="small", bufs=1) as spool,
            tc.tile_pool(name="ps1", bufs=3, space="PSUM") as ps1pool,
            tc.tile_pool(name="ps2", bufs=2, space="PSUM") as ps2pool,
        ):
            # Gating transfers for the very first matmul (x block0 k=0 and
            # w1 group0 k=0) ship first, split across the sync and scalar
            # HWDGE queues so they don't serialize behind each other.
            # Gating for the first L1 group is ~2MB (x block0 + w1 group0);
            # split it across the two HWDGE queues so each carries ~1MB and
            # the first matmuls can trickle on partial (k-sliced) data.
            xsb = {}
            xsb[0] = xpool.tile([128, 8, 512], BF16, tag="xT", name="xT0")
            w1g = []
            for g in range(8):
                t = wpool.tile([128, 2, 8, 256], BF16, tag=f"w1g{g}",
                               name=f"w1g{g}")
                w1g.append(t)
            nc.sync.dma_start(w1g[0][:, 0], w1p_d[:, 0, 0])
            nc.scalar.dma_start(xsb[0][:, :4, :], xp_d[:, 0, :4])
            nc.sync.dma_start(xsb[0][:, 4:, :], xp_d[:, 0, 4:])
            nc.scalar.dma_start(w1g[0][:, 1], w1p_d[:, 0, 1])
            if with_b1:
                b1_sb = spool.tile([128, 32], F32)
                nc.scalar.dma_start(b1_sb[:], b1_d[:])
            for g in range(1, 8):
                nc.sync.dma_start(w1g[g][:], w1p_d[:, g])
            w2g = []
            for g in range(4):
                t = wpool.tile([128, 8, 1024], BF16, tag=f"w2g{g}", name=f"w2g{g}")
                nc.sync.dma_start(t[:], w2p_d[:, g])
                w2g.append(t)

            wv_sb = spool.tile([128, capT // 128], F32)
            nc.sync.dma_start(wv_sb[:], wv_d[:])

            # HAM warm-up: the PE clock sits at 1.2GHz until it has been
            # busy ~3.4us. Spend that window on dummy matmuls while the
            # gating DMAs are in flight; sized so the burst ends right as
            # the first real tiles land (~9.5us), handing off without an
            # idle gap (an idle gap >3.4us would re-throttle the clock).
            warm_src = spool.tile([128, 128], BF16, name="warm_src")
            nc.gpsimd.memset(warm_src[:], 0.0)
            warm_ps = ps1pool.tile([128, 2, 512], F32, tag="ps1",
                                   name="warm_ps", bufs=None)
            for wi in range(52):
                nc.tensor.matmul(
                    warm_ps[:64, 0, :128], warm_src[:, :64], warm_src[:],
                    start=True, stop=True, skip_group_check=True)

            for blk in range(nblk):
                t0, tn = blk * 512, 512
                if blk not in xsb:
                    xsb[blk] = xpool.tile([128, 8, 512], BF16, tag="xT",
                                          name=f"xT{blk}")
                    nc.sync.dma_start(xsb[blk][:], xp_d[:, blk])
                xt = xsb[blk]

                # ---- layer 1: hT[m*128:(m+1)*128, :tn] for 32 H-tiles ----
                hT = hpool.tile([128, 32, 512], BF16, tag="hT", name=f"hT{blk}")
                for mg in range(16):
                    ps1 = ps1pool.tile([128, 2, 512], F32, tag="ps1",
                                       name=f"ps1_{blk}_{mg}")
                    for mj in range(2):
                        m = mg * 2 + mj
                        lg, lo = m // 4, m % 4
                        hf, co = lo // 2, (lo % 2) * 128
                        for k in range(8):
                            nc.tensor.matmul(
                                ps1[:, mj, :tn],
                                w1g[lg][:, hf, k, co:co + 128],
                                xt[:, k, :tn],
                                start=(k == 0), stop=(k == 7),
                            )
                    if with_b1:
                        for mj in range(2):
                            m = mg * 2 + mj
                            nc.scalar.activation(
                                hT[:, m, :tn], ps1[:, mj, :tn],
                                mybir.ActivationFunctionType.Gelu,
                                bias=b1_sb[:, m:m + 1],
                            )
                    else:
                        nc.scalar.activation(
                            hT[:, mg * 2:mg * 2 + 2, :tn], ps1[:, :, :tn],
                            mybir.ActivationFunctionType.Gelu,
                        )

                # ---- layer 2: y[t0+tm*128 ..., :] = hT.T @ W2, scaled ----
                for tm in range(tn // 128):
                    col = t0 // 128 + tm
                    rows = slice(t0 + tm * 128, t0 + (tm + 1) * 128)
                    for dn in range(2):
                        ps2 = ps2pool.tile([128, 512], F32, tag="ps2",
                                           name=f"ps2_{blk}_{tm}_{dn}")
                        for h in range(32):
                            nc.tensor.matmul(
                                ps2[:, :],
                                hT[:, h, tm * 128:(tm + 1) * 128],
                                w2g[h // 8][:, h % 8, dn * 512:(dn + 1) * 512],
                                start=(h == 0), stop=(h == 31),
                            )
                        yt = ypool.tile([128, 512], BF16, tag="yt",
                                        name=f"yt_{blk}_{tm}_{dn}")
                        nc.vector.tensor_scalar_mul(
                            yt[:], ps2[:], wv_sb[:, col:col + 1])
                        if blk == nblk - 1 and dn == 1:
                            nc.scalar.dma_start(
                                y_d[rows, 512:1024], yt[:])
                        else:
                            nc.sync.dma_start(
                                y_d[rows, dn * 512:(dn + 1) * 512], yt[:])

    nc.compile()
    return nc


def _route(x_flat, Wg, bg):
    """Host gate: returns per-expert (token_idx, combine_weight)."""
    logits = x_flat @ Wg.astype(np.float32) + bg.astype(np.float32)
    T = logits.shape[0]
    ar = np.arange(T)
    top1 = np.argmax(logits, axis=1)
    l2 = logits.copy()
    l2[ar, top1] = -np.inf
    top2 = np.argmax(l2, axis=1)
    v1 = logits[ar, top1]
    v2 = logits[ar, top2]
    # softmax over the two selected logits (v1 >= v2)
    e2 = np.exp(v2 - v1)
    s = 1.0 + e2
    wt1 = (1.0 / s).astype(np.float32)
    wt2 = (e2 / s).astype(np.float32)
    idx, wgt = [], []
    for e in range(E):
        m1 = top1 == e
        m2 = top2 == e
        ii = np.concatenate([ar[m1], ar[m2]])
        ww = np.concatenate([wt1[m1], wt2[m2]])
        order = np.argsort(ii, kind="stable")
        idx.append(ii[order])
        wgt.append(ww[order])
    return idx, wgt


def kernel(x, Wg, bg, W1, b1, W2, b2, _trace=None):
    global LAST_RESULTS
    x = np.asarray(x, dtype=np.float32)
    Wg = np.asarray(Wg, dtype=np.float32)
    bg = np.asarray(bg, dtype=np.float32)
    W1 = np.asarray(W1, dtype=np.float32)
    b1 = np.asarray(b1, dtype=np.float32)
    W2 = np.asarray(W2, dtype=np.float32)
    b2 = np.asarray(b2, dtype=np.float32)

    B, S, _D = x.shape
    T = B * S
    x_flat = np.ascontiguousarray(x.reshape(T, _D))

    idx, wgt = _route(x_flat, Wg, bg)
    counts = [len(i) for i in idx]
    # Fixed capacity = the load-balanced share (capacity factor 1.0).
    # Every core computes capT tokens regardless, so padding all cores to
    # the max expert count just burns PE time on zeros; overflow beyond
    # capT takes the exact host path instead.
    capT = min(2048, max(512, -(-max(counts) // 512) * 512))
    dev_counts = [min(c, capT) for c in counts]
    nblk = capT // 512

    with_b1 = bool(np.any(b1))
    key = (capT, with_b1)
    if key not in _CACHE:
        _CACHE[key] = _build(capT, with_b1)
    nc = _CACHE[key]

    bf = ml_dtypes.bfloat16
    in_maps = []
    for e in range(E):
        cnt = dev_counts[e]
        xT = np.zeros((D, capT), dtype=bf)
        if cnt:
            xT[:, :cnt] = x_flat[idx[e][:cnt]].T
        xp = xT.reshape(8, 128, nblk, 512).transpose(1, 2, 0, 3)
        w1p = W1[e].astype(bf).reshape(8, 128, 8, 2, 256).transpose(
            1, 2, 3, 0, 4)
        w2p = W2[e].astype(bf).reshape(4, 8, 128, 1024).transpose(2, 0, 1, 3)
        wv = np.zeros((capT // 128, 128), dtype=np.float32)
        if cnt:
            wv.reshape(-1)[:cnt] = wgt[e][:cnt]
        m = {
            "xp": np.ascontiguousarray(xp),
            "w1p": np.ascontiguousarray(w1p),
            "w2p": np.ascontiguousarray(w2p),
            "wv": np.ascontiguousarray(wv.T),
        }
        if with_b1:
            m["b1t"] = np.ascontiguousarray(b1[e].reshape(32, 128).T)
        in_maps.append(m)

    do_trace = TRACE if _trace is None else _trace
    res = run_bass_kernel_spmd(nc, in_maps, list(range(N_CORES)),
                               trace=do_trace)
    LAST_RESULTS = res

    out = np.zeros((T, D), dtype=np.float32)
    for e in range(E):
        cnt = dev_counts[e]
        if not cnt:
            continue
        ye = res.results[e]["y"][:cnt].astype(np.float32)
        if np.any(b2[e]):
            ye = ye + np.outer(wgt[e][:cnt], b2[e])
        out[idx[e][:cnt]] += ye

    # Host overflow path: exact fp32 FFN for tokens beyond expert capacity.
    for e in range(E):
        if counts[e] <= capT:
            continue
        oi = idx[e][capT:]
        ow = wgt[e][capT:]
        h = x_flat[oi] @ W1[e] + b1[e]
        h = 0.5 * h * (1.0 + _erf(h * np.float32(0.7071067811865476)))
        ye = h @ W2[e] + b2[e]
        out[oi] += ow[:, None] * ye

    return out.reshape(B, S, D)
